# revision 1
# baseline (speedup 1.0000x reference)
"""Trainium2 Bass kernel for ragged KeyQueryAttention pooling.

Math (per batch b):
    logits[t] = sum_l (x @ K)[t,l] * (x @ Q)[t,l],   t < len_b
    att = softmax(logits over valid t)
    out[b]    = sum_t att[t] * x[t, :] + bias        (sum att == 1)

Device strategy (8 NeuronCores, data-parallel over batch):
  - B=64 batches sorted by length (desc), grouped into 8 slots of 8;
    core i takes batch rank 8*j+i for slot j. All cores share one SPMD
    program whose per-slot chunk counts n_j = ceil(max_group_len/128)
    are compiled from the actual lengths (value-specialized; rebuilt per
    call). Rows past each batch's length are masked with -1e30.
  - Per 128-row chunk: TensorE transpose -> xT; matmul xT.T @ [K|Q]
    -> G=[keys|queries]; fused VectorE multiply+reduce -> logits column.
  - Per slot: masked softmax on [128, n_j] logits (max via TensorE
    transpose + broadcast matmul, exp on ScalarE with accum_out giving
    the row-sums for Z), then weighted-sum matmuls with the attention
    column as the stationary operand accumulating [1,128] in PSUM.
  - Host: out = acc/Z + bias, un-permute batches.
"""

import os
import numpy as np

import concourse.bass as bass
import concourse.bacc as bacc
import concourse.tile as tile
from concourse import mybir
from concourse.bass_utils import run_bass_kernel_spmd
from concourse.masks import make_identity

B, T, D, L = 64, 8192, 128, 64
NCORES = 8
SLOTS = B // NCORES  # 8 slots per core
F32 = mybir.dt.float32

LAST_EXEC_NS = None  # filled when KQA_TRACE=1

_PROG_CACHE = {}


def _build_program(n_list):
    nc = bacc.Bacc()
    ntot = sum(n_list)
    xs = [
        nc.declare_dram_parameter(f"x{j}", [n * 128, D], F32, isOutput=False)
        for j, n in enumerate(n_list)
    ]
    kq = nc.declare_dram_parameter("kq", [D, 2 * L], F32, isOutput=False)
    maskp = nc.declare_dram_parameter("mask", [128, ntot], F32, isOutput=False)
    outp = nc.declare_dram_parameter("out", [1, SLOTS * (D + 1)], F32, isOutput=True)

    AF = mybir.ActivationFunctionType
    ALU = mybir.AluOpType

    with tile.TileContext(nc) as tc:
        with (
            tc.tile_pool(name="consts", bufs=1) as consts,
            tc.tile_pool(name="xpool", bufs=3) as xpool,
            tc.tile_pool(name="spool", bufs=6) as spool,
            tc.tile_pool(name="psA", bufs=2, space="PSUM") as psA,
            tc.tile_pool(name="psB", bufs=1, space="PSUM") as psB,
        ):
            identity = consts.tile([128, 128], F32)
            make_identity(nc, identity)
            kq_sb = consts.tile([D, 2 * L], F32)
            nc.sync.dma_start(out=kq_sb, in_=kq[:, :])
            ones_col = consts.tile([128, 1], F32)
            nc.vector.memset(ones_col, 1.0)
            neg_row = consts.tile([1, 128], F32)
            nc.vector.memset(neg_row, -1.0)
            mask_sb = consts.tile([128, ntot], F32)
            nc.sync.dma_start(out=mask_sb, in_=maskp[:, :])
            out_sb = consts.tile([1, SLOTS * (D + 1)], F32)

            off = 0
            for j, n in enumerate(n_list):
                # whole slot load: [n*128, D] -> [128, n, D] (512B rows)
                x_sb = xpool.tile([128, n, D], F32, tag="x")
                for c in range(n):
                    nc.sync.dma_start(
                        out=x_sb[:, c, :], in_=xs[j][c * 128 : (c + 1) * 128, :]
                    )
                logits = spool.tile([128, n], F32, tag="logits")
                for c in range(n):
                    xT_ps = psA.tile([128, 128], F32, tag="xT")
                    nc.tensor.transpose(xT_ps, x_sb[:, c, :], identity)
                    xT_sb = spool.tile([128, 128], F32, tag="xTs")
                    nc.vector.tensor_copy(xT_sb, xT_ps)
                    g_ps = psA.tile([128, 2 * L], F32, tag="g")
                    nc.tensor.matmul(g_ps, xT_sb, kq_sb, start=True, stop=True)
                    scr = spool.tile([128, L], F32, tag="scr")
                    s1 = spool.tile([128, 1], F32, tag="s1")
                    nc.scalar.activation(
                        scr, g_ps[:, 0:L], AF.Square, accum_out=s1
                    )
                    scr2 = spool.tile([128, L], F32, tag="scr2")
                    s2 = spool.tile([128, 1], F32, tag="s2")
                    nc.scalar.activation(
                        scr2, g_ps[:, L : 2 * L], AF.Square, accum_out=s2
                    )
                    nc.vector.tensor_sub(logits[:, c : c + 1], s1, s2)
                # ragged mask (additive -1e30 on invalid rows)
                nc.vector.tensor_tensor(
                    logits, logits, mask_sb[:, off : off + n], op=ALU.add
                )
                # global max over [128, n]: free-dim reduce, transpose, reduce
                rowmax = spool.tile([128, 1], F32, tag="rmax")
                nc.vector.tensor_reduce(
                    rowmax, logits, axis=mybir.AxisListType.X, op=ALU.max
                )
                rmT_ps = psB.tile([1, 128], F32, tag="rmT")
                nc.tensor.transpose(rmT_ps, rowmax, identity)
                maxs = spool.tile([1, 1], F32, tag="maxs")
                nc.vector.tensor_reduce(
                    maxs, rmT_ps, axis=mybir.AxisListType.X, op=ALU.max
                )
                # broadcast -max to all partitions: (-1s)[1,128].T @ max[1,1]
                negm_ps = psB.tile([128, 1], F32, tag="negm")
                nc.tensor.matmul(negm_ps, neg_row, maxs, start=True, stop=True)
                negm_sb = spool.tile([128, 1], F32, tag="negms")
                nc.vector.tensor_copy(negm_sb, negm_ps)
                # P = exp(logits - max); zrow = per-partition sum of P
                p_sb = spool.tile([128, n], F32, tag="p")
                zrow = spool.tile([128, 1], F32, tag="zrow")
                nc.scalar.activation(
                    p_sb, logits, AF.Exp, bias=negm_sb, scale=1.0, accum_out=zrow
                )
                # weighted sum: acc[1,128] += P_col.T @ x_chunk
                wacc_ps = psB.tile([1, D], F32, tag="wacc")
                for c in range(n):
                    nc.tensor.matmul(
                        wacc_ps,
                        p_sb[:, c : c + 1],
                        x_sb[:, c, :],
                        start=(c == 0),
                        stop=(c == n - 1),
                    )
                z_ps = psB.tile([1, 1], F32, tag="z")
                nc.tensor.matmul(z_ps, zrow, ones_col, start=True, stop=True)
                base = j * (D + 1)
                nc.scalar.activation(out_sb[0:1, base : base + D], wacc_ps, AF.Copy)
                nc.scalar.activation(
                    out_sb[0:1, base + D : base + D + 1], z_ps, AF.Copy
                )
                off += n
            nc.sync.dma_start(out=outp[:, :], in_=out_sb)
    nc.finalize()
    return nc


def kernel(seq, lengths, key_w, query_w, bias):
    global LAST_EXEC_NS
    seq = np.asarray(seq, dtype=np.float32)
    lengths_np = np.asarray(lengths).astype(np.int64)
    key_w = np.asarray(key_w, dtype=np.float32)
    query_w = np.asarray(query_w, dtype=np.float32)
    bias = np.asarray(bias, dtype=np.float32)

    order = np.argsort(-lengths_np, kind="stable")  # descending length
    n_list = []
    for j in range(SLOTS):
        grp = order[j * NCORES : (j + 1) * NCORES]
        n_list.append(max(1, int(-(-int(lengths_np[grp].max()) // 128))))
    key = tuple(n_list)
    if key not in _PROG_CACHE:
        _PROG_CACHE[key] = _build_program(n_list)
    nc = _PROG_CACHE[key]

    kqcat = np.concatenate(
        [(key_w + query_w) * 0.5, (key_w - query_w) * 0.5], axis=1
    )  # [D, 2L]; logits = ||x@U1||^2 - ||x@U2||^2 = rowwise dot of x@K and x@Q
    in_maps = []
    for i in range(NCORES):
        m = {"kq": kqcat}
        mask_cols = []
        for j, n in enumerate(n_list):
            b = int(order[j * NCORES + i])
            m[f"x{j}"] = seq[b, : n * 128, :]
            lb = int(lengths_np[b])
            col = np.where(np.arange(n * 128) < lb, 0.0, -1e30).astype(np.float32)
            mask_cols.append(col.reshape(n, 128).T)  # [128, n]
        m["mask"] = np.ascontiguousarray(np.concatenate(mask_cols, axis=1))
        in_maps.append(m)

    trace = os.environ.get("KQA_TRACE") == "1"
    res = run_bass_kernel_spmd(
        nc, in_maps, core_ids=list(range(NCORES)), trace=trace
    )
    LAST_EXEC_NS = res.exec_time_ns

    out = np.empty((B, D), dtype=np.float32)
    for i in range(NCORES):
        r = res.results[i]["out"].reshape(SLOTS, D + 1)
        for j in range(SLOTS):
            b = int(order[j * NCORES + i])
            acc = r[j, :D]
            z = r[j, D]
            out[b] = acc / z + bias
    return out



# revision 2
# speedup vs baseline: 4.3417x; 4.3417x over previous
"""Trainium2 Bass kernel for ragged KeyQueryAttention pooling.

Math (per batch b):
    logits[t] = sum_l (x @ K)[t,l] * (x @ Q)[t,l],   t < len_b
    att = softmax(logits over valid t)
    out[b]    = sum_t att[t] * x[t, :] + bias        (sum att == 1)

Device strategy (8 NeuronCores, data-parallel over batch):
  - B=64 batches sorted by length (desc), grouped into 8 slots of 8;
    core i takes batch rank 8*j+i for slot j. One SPMD program whose
    per-slot chunk counts n_j = ceil(max_group_len/128) are compiled
    from the actual lengths (value-specialized, cached per n-tuple).
  - Host casts seq to fp16 and pre-arranges each core's slots into a
    single [128, ntot*128] image (partition = t%128, free = chunk,d),
    halving HBM traffic and giving the DMA large contiguous lines.
    Host also folds K,Q into kq = [(K+Q)/2 | (K-Q)/2] fp16 so that
    logits = ||x@U1||^2 - ||x@U2||^2 per row (diff of squares).
  - Per 8-chunk group: TensorE fp16 transposes (PSUM fp16), one DVE
    2x copy PSUM->SBUF, 8 fp16 matmuls -> g = x@[U1|U2] (PSUM fp32),
    one ScalarE Square (PSUM->SBUF), one GpSimd subtract of halves,
    one DVE reduce -> logits columns [128, w]. Everything batched to
    amortize per-instruction fixed overheads.
  - Per slot: GpSimd ragged mask add (-1e30), DVE row max, TensorE
    transpose + DVE reduce + broadcast matmul -> -max, ScalarE exp
    (bias=-max) -> p fp16 with fp32 row sums (zrow) as accum_out,
    then n accumulating matmuls (lhsT = x chunk, moving = p column)
    -> weighted sum [128(d), 1] in PSUM.
  - Output [128, 2*SLOTS] fp32: cols j = wsum_j, cols 8+j = zrow_j.
    Host: out[b] = wsum/sum(zrow) + bias, un-permute batches.
"""

import os
import numpy as np

import concourse.bass as bass
import concourse.bacc as bacc
import concourse.tile as tile
from concourse import mybir
from concourse.bass_utils import run_bass_kernel_spmd
from concourse.masks import make_identity

B, T, D, L = 64, 8192, 128, 64
NCORES = 8
SLOTS = B // NCORES
F32 = mybir.dt.float32
F16 = mybir.dt.float16
G = 8  # chunks per instruction group

LAST_EXEC_NS = None  # filled when KQA_TRACE=1

_PROG_CACHE = {}


def _build_program(n_list):
    nc = bacc.Bacc()
    ntot = sum(n_list)
    offs = [sum(n_list[:j]) for j in range(SLOTS)]

    X = nc.declare_dram_parameter("X", [128, ntot * 128], F16, isOutput=False)
    kq = nc.declare_dram_parameter("kq", [D, 2 * L], F16, isOutput=False)
    maskp = nc.declare_dram_parameter("mask", [128, ntot], F32, isOutput=False)
    outp = nc.declare_dram_parameter("out", [128, 2 * SLOTS], F32, isOutput=True)

    AF = mybir.ActivationFunctionType
    ALU = mybir.AluOpType
    AX = mybir.AxisListType

    with tile.TileContext(nc) as tc:
        with (
            tc.tile_pool(name="consts", bufs=1) as consts,
            tc.tile_pool(name="xgp", bufs=16) as xgp,
            tc.tile_pool(name="work", bufs=3) as work,
            tc.tile_pool(name="slotp", bufs=2) as slotp,
            tc.tile_pool(name="psT", bufs=2, space="PSUM") as psT,
            tc.tile_pool(name="psG", bufs=2, space="PSUM") as psG,
            tc.tile_pool(name="psM", bufs=2, space="PSUM") as psM,
        ):
            identity16 = consts.tile([128, 128], F16)
            make_identity(nc, identity16)
            identity32 = consts.tile([128, 128], F32)
            make_identity(nc, identity32)
            kq_sb = consts.tile([D, 2 * L], F16)
            nc.sync.dma_start(out=kq_sb, in_=kq[:, :])
            mask_sb = consts.tile([128, ntot], F32)
            nc.sync.dma_start(out=mask_sb, in_=maskp[:, :])
            negrow = consts.tile([1, 128], F32)
            nc.vector.memset(negrow, -1.0)
            logits = consts.tile([128, ntot], F32)
            out_sb = consts.tile([128, 2 * SLOTS], F32)

            ngroups = [-(-n // G) for n in n_list]
            xg_tiles = [[] for _ in range(SLOTS)]

            def emit_dma(j):
                n, off = n_list[j], offs[j]
                for k in range(ngroups[j]):
                    c0 = k * G
                    w = min(G, n - c0)
                    xg = xgp.tile([128, G, 128], F16, tag="xg", name=f"xg{j}_{k}")
                    nc.sync.dma_start(
                        out=xg[:, 0:w, :],
                        in_=X[:, (off + c0) * 128 : (off + c0 + w) * 128],
                    )
                    xg_tiles[j].append(xg)

            def emit_A(j):
                n, off = n_list[j], offs[j]
                for k in range(ngroups[j]):
                    c0 = k * G
                    w = min(G, n - c0)
                    xg = xg_tiles[j][k]
                    xT_ps = psT.tile([128, G, 128], F16, tag="xT")
                    for i in range(w):
                        nc.tensor.transpose(xT_ps[:, i, :], xg[:, i, :], identity16)
                    xT_sb = work.tile([128, G, 128], F16, tag="xTs")
                    nc.vector.tensor_copy(xT_sb[:, 0:w, :], xT_ps[:, 0:w, :])
                    g_ps = psG.tile([128, G, 128], F32, tag="g")
                    for i in range(w):
                        nc.tensor.matmul(
                            g_ps[:, i, :], xT_sb[:, i, :], kq_sb, start=True, stop=True
                        )
                    sq = work.tile([128, G, 128], F32, tag="sq")
                    nc.scalar.activation(sq[:, 0:w, :], g_ps[:, 0:w, :], AF.Square)
                    dd = work.tile([128, G, L], F32, tag="dd")
                    nc.gpsimd.tensor_tensor(
                        dd[:, 0:w, :],
                        sq[:, 0:w, 0:L],
                        sq[:, 0:w, L : 2 * L],
                        op=ALU.subtract,
                    )
                    nc.vector.tensor_reduce(
                        logits[:, off + c0 : off + c0 + w],
                        dd[:, 0:w, :],
                        axis=AX.X,
                        op=ALU.add,
                    )

            def emit_B(j):
                n, off = n_list[j], offs[j]
                lm = slotp.tile([128, 64], F32, tag="lm")
                nc.gpsimd.tensor_tensor(
                    lm[:, 0:n],
                    logits[:, off : off + n],
                    mask_sb[:, off : off + n],
                    op=ALU.add,
                )
                rowmax = slotp.tile([128, 1], F32, tag="rmax")
                nc.vector.tensor_reduce(rowmax, lm[:, 0:n], axis=AX.X, op=ALU.max)
                misc = psM.tile([128, 256], F32, tag="misc")
                nc.tensor.transpose(misc[0:1, 128:256], rowmax, identity32)
                maxs = slotp.tile([1, 1], F32, tag="maxs")
                nc.vector.tensor_reduce(
                    maxs, misc[0:1, 128:256], axis=AX.X, op=ALU.max
                )
                nc.tensor.matmul(misc[:, 1:2], negrow, maxs, start=True, stop=True)
                negm_sb = slotp.tile([128, 1], F32, tag="negm")
                nc.scalar.activation(negm_sb, misc[:, 1:2], AF.Copy)
                p_sb = slotp.tile([128, 64], F16, tag="p")
                nc.scalar.activation(
                    p_sb[:, 0:n],
                    lm[:, 0:n],
                    AF.Exp,
                    bias=negm_sb,
                    scale=1.0,
                    accum_out=out_sb[:, SLOTS + j : SLOTS + j + 1],
                )
                for c in range(n):
                    nc.tensor.matmul(
                        misc[:, 0:1],
                        xg_tiles[j][c // G][:, c % G, :],
                        p_sb[:, c : c + 1],
                        start=(c == 0),
                        stop=(c == n - 1),
                    )
                nc.scalar.activation(out_sb[:, j : j + 1], misc[:, 0:1], AF.Copy)

            emit_dma(0)
            if SLOTS > 1:
                emit_dma(1)
            for j in range(SLOTS):
                emit_A(j)
                if j >= 1:
                    emit_B(j - 1)
                    if j + 1 < SLOTS:
                        emit_dma(j + 1)
            emit_B(SLOTS - 1)
            nc.sync.dma_start(out=outp[:, :], in_=out_sb)
    nc.finalize()
    return nc


def kernel(seq, lengths, key_w, query_w, bias):
    global LAST_EXEC_NS
    seq = np.asarray(seq, dtype=np.float32)
    lengths_np = np.asarray(lengths).astype(np.int64)
    key_w = np.asarray(key_w, dtype=np.float32)
    query_w = np.asarray(query_w, dtype=np.float32)
    bias = np.asarray(bias, dtype=np.float32)

    order = np.argsort(-lengths_np, kind="stable")  # descending length
    n_list = []
    for j in range(SLOTS):
        grp = order[j * NCORES : (j + 1) * NCORES]
        n_list.append(max(1, int(-(-int(lengths_np[grp].max()) // 128))))
    key = tuple(n_list)
    if key not in _PROG_CACHE:
        _PROG_CACHE[key] = _build_program(n_list)
    nc = _PROG_CACHE[key]

    ntot = sum(n_list)
    seq16 = seq.astype(np.float16)
    kqcat = np.concatenate(
        [(key_w + query_w) * 0.5, (key_w - query_w) * 0.5], axis=1
    ).astype(np.float16)

    in_maps = []
    for i in range(NCORES):
        xblocks = []
        mblocks = []
        for j, n in enumerate(n_list):
            b = int(order[j * NCORES + i])
            blk = seq16[b, : n * 128, :].reshape(n, 128, 128).transpose(1, 0, 2)
            xblocks.append(blk.reshape(128, n * 128))
            lb = int(lengths_np[b])
            col = np.where(np.arange(n * 128) < lb, 0.0, -1e30).astype(np.float32)
            mblocks.append(col.reshape(n, 128).T)
        in_maps.append(
            {
                "X": np.ascontiguousarray(np.concatenate(xblocks, axis=1)),
                "kq": kqcat,
                "mask": np.ascontiguousarray(np.concatenate(mblocks, axis=1)),
            }
        )

    trace = os.environ.get("KQA_TRACE") == "1"
    res = run_bass_kernel_spmd(
        nc, in_maps, core_ids=list(range(NCORES)), trace=trace
    )
    LAST_EXEC_NS = res.exec_time_ns

    out = np.empty((B, D), dtype=np.float32)
    for i in range(NCORES):
        r = res.results[i]["out"]  # [128, 2*SLOTS]
        for j in range(SLOTS):
            b = int(order[j * NCORES + i])
            z = r[:, SLOTS + j].astype(np.float64).sum()
            out[b] = (r[:, j] / z).astype(np.float32) + bias
    return out
